# revision 16
# baseline (speedup 1.0000x reference)
"""GraphSAGE-style pooling aggregator kernel for Trainium2 (8 NeuronCores).

Computes, for full inputs:
    h      = relu(neighbor_features @ w_pool + bias_pool)   # (n*k, dim)
    pooled = max(h.reshape(n, k, dim), axis=1)              # (n, dim)
    out    = relu(concat([src, pooled], -1) @ w + bias)     # (n, out)

Sharding: data-parallel over nodes. Core c gets nodes [c*1250, (c+1)*1250)
and the matching 40000 neighbor rows; weights replicated. No collectives.

On-chip dataflow (per core), all in the "transposed" domain so that the
k-neighbor max-pool is a free-dim segmented reduce:
  1. DMA neighbor rows natural [128 rows, 128 dim] (1 MB blocks)
  2. PE transpose (fp32, exact) each 128-row square -> PSUM x^T
  3. ACT copies x^T PSUM -> SBUF
  4. PE matmul: w_pool stationary, streams x^T -> PSUM h^T (no bias/relu yet:
     max_k relu(z_k + b) == relu(max_k z_k + b) since bias is per-feature)
  5. DVE segmented reduce_max over PSUM h^T [128, 16, 32] -> pooled^T columns
  6. ACT relu(pooled^T + bias_pool) (bias per-partition = per-feature)
  7. Phase 2: out[nodes,dout] = relu(src@w_top + pooled@w_bot + bias) with
     src^T / pooled^T slices as the stationary operand -> natural output rows,
     bias applied via a K=1 matmul accumulate of ones^T @ bias_row.
"""

import os

import numpy as np

N, K, DIM, OUT = 10000, 32, 128, 128
N_CORES = 8
NODES_PC = N // N_CORES          # 1250 nodes per core
ROWS_PC = NODES_PC * K           # 40000 neighbor rows per core

UNIT = 512                       # rows per PSUM/matmul/reduce unit (16 nodes)
SQ = 128                         # rows per PE-transpose square
BLOCK_UNITS = 4                  # units per DMA block (2048 rows = 1 MB)

# matmul input dtype for the two GEMMs: float32r streams 4x faster through
# the PE than float32 (single-pass relaxed-precision mode). The transposes
# always run in plain float32 (exact).
USE_F32R = os.environ.get("AGG_F32R", "1") == "1"
# debug bisection knobs
SKIP_BIAS_MM = os.environ.get("AGG_NO_BIAS_MM", "0") == "1"
SKIP_P2_MM = os.environ.get("AGG_NO_P2_MM", "0") == "1"
SKIP_P1_MM = os.environ.get("AGG_NO_P1_MM", "0") == "1"


def _build_nc():
    import concourse.bacc as bacc
    import concourse.mybir as mybir
    import concourse.tile as tile
    from concourse.masks import make_identity

    f32 = mybir.dt.float32
    f32r = mybir.dt.float32r
    # dtype of the phase-1 GEMM operands: fp32r streams 1 row/cycle through
    # the PE vs 4 for fp32. The BIR verifier requires fp32r matmul inputs to
    # be *produced* rounded (ACT copy with an f32r-typed destination).
    mmdt = f32r if USE_F32R else f32
    AX = mybir.AxisListType
    AF = mybir.ActivationFunctionType

    nc = bacc.Bacc(target_bir_lowering=False)

    src = nc.declare_dram_parameter("src_features", [NODES_PC, DIM], f32, isOutput=False)
    nbr = nc.declare_dram_parameter("neighbor_features", [ROWS_PC, DIM], f32, isOutput=False)
    w_pool = nc.declare_dram_parameter("w_pool", [DIM, DIM], f32, isOutput=False)
    bias_pool = nc.declare_dram_parameter("bias_pool", [DIM], f32, isOutput=False)
    w = nc.declare_dram_parameter("w", [2 * DIM, OUT], f32, isOutput=False)
    bias_bc = nc.declare_dram_parameter("bias_bc", [SQ, OUT], f32, isOutput=False)
    out = nc.declare_dram_parameter("out", [NODES_PC, OUT], f32, isOutput=True)

    # unit list: (row_start, n_rows); 78x512 + 1x64
    units = []
    r = 0
    while r < ROWS_PC:
        n_r = min(UNIT, ROWS_PC - r)
        units.append((r, n_r))
        r += n_r
    blocks = [units[i : i + BLOCK_UNITS] for i in range(0, len(units), BLOCK_UNITS)]

    # src chunks for phase 2: (node_start, n_nodes); 9x128 + 1x98
    chunks = []
    c = 0
    while c < NODES_PC:
        n_c = min(SQ, NODES_PC - c)
        chunks.append((c, n_c))
        c += n_c

    with tile.TileContext(nc) as tc:
        with (
            tc.tile_pool(name="consts", bufs=1) as consts,
            tc.tile_pool(name="persist", bufs=1) as persist,
            tc.tile_pool(name="xnat", bufs=3) as xnat_pool,
            tc.tile_pool(name="xt", bufs=3) as xt_pool,
            tc.tile_pool(name="srcio", bufs=2) as srcio,
            tc.tile_pool(name="outio", bufs=2) as outio,
            tc.tile_pool(name="ps_xt", bufs=2, space="PSUM") as ps_xt,
            tc.tile_pool(name="ps_ht", bufs=2, space="PSUM") as ps_ht,
            tc.tile_pool(name="ps_p2", bufs=2, space="PSUM") as ps_p2,
            tc.tile_pool(name="ps_out", bufs=2, space="PSUM") as ps_out,
        ):
            # --- constants ---
            ident = consts.tile([SQ, SQ], f32)
            make_identity(nc, ident)
            wpool_sb = consts.tile([DIM, DIM], mmdt)
            if USE_F32R:
                wpool_st = consts.tile([DIM, DIM], f32)
                nc.sync.dma_start(out=wpool_st, in_=w_pool[:, :])
                nc.scalar.copy(out=wpool_sb, in_=wpool_st)
            else:
                nc.sync.dma_start(out=wpool_sb, in_=w_pool[:, :])
            wtop_sb = consts.tile([DIM, OUT], f32)
            nc.sync.dma_start(out=wtop_sb, in_=w[0:DIM, :])
            wbot_sb = consts.tile([DIM, OUT], f32)
            nc.sync.dma_start(out=wbot_sb, in_=w[DIM : 2 * DIM, :])
            bpool_sb = consts.tile([DIM, 1], f32)
            nc.sync.dma_start(out=bpool_sb, in_=bias_pool.rearrange("(d o) -> d o", o=1))
            # bias broadcast across node partitions, prepared host-side
            brow_bc = consts.tile([SQ, OUT], f32)
            nc.sync.dma_start(out=brow_bc, in_=bias_bc[:, :])

            pooledT = persist.tile([DIM, NODES_PC], f32)   # raw max scores ^T
            srcT = persist.tile([DIM, len(chunks) * SQ], f32)

            # --- phase 2a: transpose src chunks (small, independent) ---
            for ci, (c0, n_c) in enumerate(chunks):
                s_nat = srcio.tile([SQ, DIM], f32)
                nc.sync.dma_start(out=s_nat[:n_c, :], in_=src[c0 : c0 + n_c, :])
                sT_ps = ps_p2.tile([DIM, SQ], f32, tag="sT")
                nc.tensor.transpose(
                    out=sT_ps[:, :n_c], in_=s_nat[:n_c, :], identity=ident[:n_c, :n_c]
                )
                nc.scalar.copy(out=srcT[:, ci * SQ : ci * SQ + n_c], in_=sT_ps[:, :n_c])

            # --- phase 1: neighbor MLP + max-pool ---
            for blk in blocks:
                r0 = blk[0][0]
                blk_rows = sum(n_r for _, n_r in blk)
                full_sq = blk_rows // SQ
                rem = blk_rows - full_sq * SQ
                n_sq = full_sq + (1 if rem else 0)

                x_nat = xnat_pool.tile([SQ, BLOCK_UNITS * UNIT // SQ, DIM], f32)
                if full_sq:
                    nc.sync.dma_start(
                        out=x_nat[:, :full_sq, :],
                        in_=nbr[r0 : r0 + full_sq * SQ].rearrange(
                            "(s p) d -> p s d", p=SQ
                        ),
                    )
                if rem:
                    nc.sync.dma_start(
                        out=x_nat[:rem, full_sq, :],
                        in_=nbr[r0 + full_sq * SQ : r0 + blk_rows],
                    )

                sq_off = 0  # square index within the block
                for u0, u_rows in blk:
                    xT_ps = ps_xt.tile([DIM, UNIT], f32, tag="xT")
                    u_sq = (u_rows + SQ - 1) // SQ
                    for s in range(u_sq):
                        sr = min(SQ, u_rows - s * SQ)
                        nc.tensor.transpose(
                            out=xT_ps[:, s * SQ : s * SQ + sr],
                            in_=x_nat[:sr, sq_off + s, :],
                            identity=ident[:sr, :sr],
                        )
                    sq_off += u_sq
                    xT_sb = xt_pool.tile([DIM, UNIT], mmdt)
                    nc.scalar.copy(out=xT_sb[:, :u_rows], in_=xT_ps[:, :u_rows])
                    hT_ps = ps_ht.tile([DIM, UNIT], f32, tag="hT")
                    if SKIP_P1_MM:
                        nc.vector.tensor_copy(
                            out=hT_ps[:, :u_rows], in_=xT_sb[:, :u_rows].bitcast(f32)
                        )
                    else:
                        nc.tensor.matmul(
                            out=hT_ps[:, :u_rows],
                            lhsT=wpool_sb[:, :],
                            rhs=xT_sb[:, :u_rows],
                            start=True,
                            stop=True,
                        )
                    n_nodes_u = u_rows // K
                    node0 = u0 // K
                    nc.vector.reduce_max(
                        out=pooledT[:, node0 : node0 + n_nodes_u],
                        in_=hT_ps[:, :u_rows].rearrange("p (n k) -> p n k", k=K),
                        axis=AX.X,
                    )

            # relu(pooled^T + bias_pool): bias is per-feature = per-partition
            nc.scalar.activation(
                out=pooledT[:, :],
                in_=pooledT[:, :],
                func=AF.Relu,
                bias=bpool_sb[:, :],
                scale=1.0,
            )

            # --- phase 2b: out = relu(src @ w_top + pooled @ w_bot + bias) ---
            for ci, (c0, n_c) in enumerate(chunks):
                o_ps = ps_out.tile([SQ, OUT], f32, tag="ops")
                if SKIP_P2_MM:
                    nc.vector.memset(o_ps[:n_c, :], 0.125)
                else:
                    nc.tensor.matmul(
                        out=o_ps[:n_c, :],
                        lhsT=srcT[:, ci * SQ : ci * SQ + n_c],
                        rhs=wtop_sb[:, :],
                        start=True,
                        stop=False,
                    )
                    nc.tensor.matmul(
                        out=o_ps[:n_c, :],
                        lhsT=pooledT[:, c0 : c0 + n_c],
                        rhs=wbot_sb[:, :],
                        start=False,
                        stop=True,
                    )
                    if not SKIP_BIAS_MM:
                        # bias is per-output-column: broadcast-add the [1, OUT]
                        # bias row across node partitions (K=1 matmuls fault)
                        nc.vector.tensor_add(
                            out=o_ps[:n_c, :],
                            in0=o_ps[:n_c, :],
                            in1=brow_bc[:n_c, :],
                        )
                o_sb = outio.tile([SQ, OUT], f32)
                nc.scalar.activation(
                    out=o_sb[:n_c, :], in_=o_ps[:n_c, :], func=AF.Relu
                )
                nc.sync.dma_start(out=out[c0 : c0 + n_c, :], in_=o_sb[:n_c, :])

    nc.compile()
    return nc


_NC_CACHE = None


def _make_in_maps(inputs):
    src = np.ascontiguousarray(inputs["src_features"], dtype=np.float32)
    nbr = np.ascontiguousarray(inputs["neighbor_features"], dtype=np.float32)
    w_pool = np.ascontiguousarray(inputs["w_pool"], dtype=np.float32)
    bias_pool = np.ascontiguousarray(inputs["bias_pool"], dtype=np.float32)
    w = np.ascontiguousarray(inputs["w"], dtype=np.float32)
    bias_bc = np.ascontiguousarray(
        np.broadcast_to(inputs["bias"], (SQ, OUT)), dtype=np.float32
    )

    in_maps = []
    for c in range(N_CORES):
        in_maps.append(
            {
                "src_features": src[c * NODES_PC : (c + 1) * NODES_PC],
                "neighbor_features": nbr[c * ROWS_PC : (c + 1) * ROWS_PC],
                "w_pool": w_pool,
                "bias_pool": bias_pool,
                "w": w,
                "bias_bc": bias_bc,
            }
        )

    return in_maps


def kernel(**inputs: np.ndarray) -> np.ndarray:
    from concourse.bass_utils import run_bass_kernel_spmd

    global _NC_CACHE
    if _NC_CACHE is None:
        _NC_CACHE = _build_nc()
    nc = _NC_CACHE

    in_maps = _make_in_maps(inputs)
    res = run_bass_kernel_spmd(nc, in_maps, core_ids=list(range(N_CORES)))
    return np.concatenate([res.results[c]["out"] for c in range(N_CORES)], axis=0)
